# revision 7
# baseline (speedup 1.0000x reference)
"""Trainium2 Bass kernel for nn_IdentityConvolution.

reference semantics:
    r = sum_c x_real[b, c, :, :]   # [B, 1, H, W]
    i = sum_c x_imag[b, c, :, :]
    out = complex(r, i) broadcast to [B, 64, H, W]  (complex64)

Sharding: data-parallel over batch B=8 across the 8 NeuronCores (one
batch image per core, no cross-core communication).

Per-core device program (fully unrolled, Tile-scheduled):
  - inputs  x_real/x_imag viewed as [C=64, P=128, Q=512] (hw = p*512+q)
  - for each q-chunk: load [128, 16, qc] tiles (full 128 partitions,
    >=1KB contiguous per descriptor), tree-add 16 channels per group on
    the DVE, accumulate 4 groups into acc_r/acc_i [128, qc]
  - interleave acc_r/acc_i into an SBUF tile [128, 2*qc] matching the
    complex64 byte layout
  - DMA that tile to all 64 output-channel planes (contiguous blocks)
"""

import sys

sys.path.insert(0, "/opt/trn_rl_repo")

from contextlib import ExitStack

import numpy as np

import concourse.bass as bass
import concourse.bacc as bacc
import concourse.tile as tile
from concourse import mybir
from concourse.bass_utils import run_bass_kernel_spmd

B, C, H, W = 8, 64, 256, 256
P = 128
Q = (H * W) // P  # 512
NCG = 4  # channel groups
CG = C // NCG  # 16 channels per group
NHW = 2  # q chunks
QC = Q // NHW  # 256

F32 = mybir.dt.float32

_cache = {}


def _build_program():
    nc = bacc.Bacc("TRN2", target_bir_lowering=False, debug=False, num_devices=8)
    xr = nc.dram_tensor("x_real", [C, P, Q], F32, kind="ExternalInput").ap()
    xi = nc.dram_tensor("x_imag", [C, P, Q], F32, kind="ExternalInput").ap()
    out = nc.dram_tensor("out", [C, P, 2 * Q], F32, kind="ExternalOutput").ap()

    xr_v = xr.rearrange("c p q -> p c q")
    xi_v = xi.rearrange("c p q -> p c q")

    with tile.TileContext(nc) as tc, ExitStack() as ctx:
        inp = ctx.enter_context(tc.tile_pool(name="inp", bufs=4))
        scr = ctx.enter_context(tc.tile_pool(name="scr", bufs=2))
        accp = ctx.enter_context(tc.tile_pool(name="acc", bufs=2))
        outp = ctx.enter_context(tc.tile_pool(name="outp", bufs=2))

        for j in range(NHW):
            q0 = j * QC
            acc_r = accp.tile([P, QC], F32, tag="acc_r")
            acc_i = accp.tile([P, QC], F32, tag="acc_i")
            for x_v, acc in ((xr_v, acc_r), (xi_v, acc_i)):
                for g in range(NCG):
                    t = inp.tile([P, CG, QC], F32, tag="in")
                    nc.sync.dma_start(
                        out=t[:],
                        in_=x_v[:, g * CG : (g + 1) * CG, q0 : q0 + QC],
                    )
                    # first tree level reads the big tile exactly once so
                    # the slot's next DMA writer has few sync waits
                    s = scr.tile([P, CG // 2, QC], F32, tag="s")
                    h = CG // 2
                    nc.vector.tensor_add(s[:], t[:, 0:h, :], t[:, h:CG, :])
                    m = h
                    while m > 1:
                        m //= 2
                        nc.vector.tensor_add(
                            s[:, 0:m, :], s[:, 0:m, :], s[:, m : 2 * m, :]
                        )
                    top = s[:, 0, :]
                    if g == 0:
                        nc.vector.tensor_copy(acc[:], top)
                    else:
                        nc.vector.tensor_add(acc[:], acc[:], top)

            ot = outp.tile([P, 2 * QC], F32, tag="ot")
            otv = ot[:].rearrange("p (q t) -> p q t", t=2)
            nc.vector.tensor_copy(otv[:, :, 0], acc_r[:])
            nc.vector.tensor_copy(otv[:, :, 1], acc_i[:])
            for co in range(C):
                nc.sync.dma_start(
                    out=out[co, :, 2 * q0 : 2 * q0 + 2 * QC], in_=ot[:]
                )
    nc.compile()
    return nc


def kernel(x_real, x_imag, _profile=False):
    if "nc" not in _cache:
        _cache["nc"] = _build_program()
    nc = _cache["nc"]

    x_real = np.asarray(x_real)
    x_imag = np.asarray(x_imag)
    in_maps = [
        {
            "x_real": np.ascontiguousarray(x_real[b]).reshape(C, P, Q),
            "x_imag": np.ascontiguousarray(x_imag[b]).reshape(C, P, Q),
        }
        for b in range(B)
    ]
    res = run_bass_kernel_spmd(nc, in_maps, list(range(B)), trace=_profile)
    _cache["last_result"] = res

    out = np.empty((B, C, H, W), dtype=np.complex64)
    for b in range(B):
        o = res.results[b]["out"]  # [C, P, 2Q] f32
        out[b] = o.reshape(C, P * Q, 2).view(np.complex64).reshape(C, H, W)
    return out


# revision 9
# speedup vs baseline: 19.9788x; 19.9788x over previous
"""Trainium2 Bass kernel for nn_IdentityConvolution.

reference semantics:
    r = sum_c x_real[b, c, :, :]   # [B, 1, H, W]
    i = sum_c x_imag[b, c, :, :]
    out = complex(r, i) broadcast to [B, 64, H, W]  (complex64)

Sharding: data-parallel over batch B=8 across the 8 NeuronCores (one
batch image per core, no cross-core communication).

Per-core device program (fully unrolled, Tile-scheduled):
  - inputs  x_real/x_imag viewed as [C=64, P=128, Q=512] (hw = p*512+q)
  - for each q-chunk: load [128, 16, qc] tiles (full 128 partitions,
    >=1KB contiguous per descriptor), tree-add 16 channels per group on
    the DVE, accumulate 4 groups into acc_r/acc_i [128, qc]
  - interleave acc_r/acc_i into an SBUF tile [128, 2*qc] matching the
    complex64 byte layout
  - DMA that tile to all 64 output-channel planes (contiguous blocks)
"""

import sys

sys.path.insert(0, "/opt/trn_rl_repo")

from contextlib import ExitStack

import numpy as np

import concourse.bass as bass
import concourse.bacc as bacc
import concourse.tile as tile
from concourse import mybir
from concourse.bass_utils import run_bass_kernel_spmd

B, C, H, W = 8, 64, 256, 256
P = 128
Q = (H * W) // P  # 512
NCG = 4  # channel groups
CG = C // NCG  # 16 channels per group
NHW = 2  # q chunks
QC = Q // NHW  # 256

F32 = mybir.dt.float32

_cache = {}


def _build_program(repeat=1):
    nc = bacc.Bacc("TRN2", target_bir_lowering=False, debug=False, num_devices=8)
    xr = nc.dram_tensor("x_real", [C, P, Q], F32, kind="ExternalInput").ap()
    xi = nc.dram_tensor("x_imag", [C, P, Q], F32, kind="ExternalInput").ap()
    out = nc.dram_tensor("out", [C, P, 2 * Q], F32, kind="ExternalOutput").ap()

    xr_v = xr.rearrange("c p q -> p c q")
    xi_v = xi.rearrange("c p q -> p c q")

    with tile.TileContext(nc) as tc, ExitStack() as ctx:
        inp = ctx.enter_context(tc.tile_pool(name="inp", bufs=4))
        scr = ctx.enter_context(tc.tile_pool(name="scr", bufs=2))
        accp = ctx.enter_context(tc.tile_pool(name="acc", bufs=2))
        outp = ctx.enter_context(tc.tile_pool(name="outp", bufs=2))

        for j in range(NHW * repeat):
            j = j % NHW
            q0 = j * QC
            acc_r = accp.tile([P, QC], F32, tag="acc_r")
            acc_i = accp.tile([P, QC], F32, tag="acc_i")
            for x_v, acc in ((xr_v, acc_r), (xi_v, acc_i)):
                for g in range(NCG):
                    t = inp.tile([P, CG, QC], F32, tag="in")
                    nc.sync.dma_start(
                        out=t[:],
                        in_=x_v[:, g * CG : (g + 1) * CG, q0 : q0 + QC],
                    )
                    # first tree level reads the big tile exactly once so
                    # the slot's next DMA writer has few sync waits
                    s = scr.tile([P, CG // 2, QC], F32, tag="s")
                    h = CG // 2
                    nc.vector.tensor_add(s[:], t[:, 0:h, :], t[:, h:CG, :])
                    m = h
                    while m > 1:
                        m //= 2
                        nc.vector.tensor_add(
                            s[:, 0:m, :], s[:, 0:m, :], s[:, m : 2 * m, :]
                        )
                    top = s[:, 0, :]
                    if g == 0:
                        nc.vector.tensor_copy(acc[:], top)
                    else:
                        nc.vector.tensor_add(acc[:], acc[:], top)

            ot = outp.tile([P, 2 * QC], F32, tag="ot")
            otv = ot[:].rearrange("p (q t) -> p q t", t=2)
            nc.vector.tensor_copy(otv[:, :, 0], acc_r[:])
            nc.vector.tensor_copy(otv[:, :, 1], acc_i[:])
            for co in range(C):
                nc.sync.dma_start(
                    out=out[co, :, 2 * q0 : 2 * q0 + 2 * QC], in_=ot[:]
                )
    nc.compile()
    return nc


def kernel(x_real, x_imag, _profile=False):
    if "nc" not in _cache:
        _cache["nc"] = _build_program()
    nc = _cache["nc"]

    x_real = np.asarray(x_real)
    x_imag = np.asarray(x_imag)
    in_maps = [
        {
            "x_real": np.ascontiguousarray(x_real[b]).reshape(C, P, Q),
            "x_imag": np.ascontiguousarray(x_imag[b]).reshape(C, P, Q),
        }
        for b in range(B)
    ]
    res = run_bass_kernel_spmd(nc, in_maps, list(range(B)), trace=_profile)
    _cache["last_result"] = res

    out = np.empty((B, C, H, W), dtype=np.complex64)
    for b in range(B):
        o = res.results[b]["out"]  # [C, P, 2Q] f32
        out[b] = o.reshape(C, P * Q, 2).view(np.complex64).reshape(C, H, W)
    return out
